# revision 19
# baseline (speedup 1.0000x reference)
"""Trainium2 Bass kernel for nn_CombinatorialClassifier (v4).

Computation (reference):
    logits = einsum('bf,pqf->bpq', x, W) + b        # [B,P,Q]
    logp   = log_softmax(logits, axis=2)            # [B,P,Q]
    out    = take_along_axis(logp, part_idx, 2)     # [B,P,C]

Shapes: B=256, P=64, Q=128, C=1000, F=2048.  Expert-parallel over P
across 8 cores (PL=8 partitionings per core), no collectives.

Structure:
  - main matmul in "b-orientation": stationary = xT k-slab [128f, 128b]
    (fp8e4), moving = W k-slab [128f, (p,q)-chunk] (fp8e4, x64 scale
    folded back out in the softmax) -> psum_lin[b, (p,q)].  W streams
    from HBM in 4 chunks on the sync HWDGE family; idx streams in
    parallel on the scalar HWDGE family so it does not delay W.
  - PE HAM warm-up: junk K=1 matmuls at t=0 off the tiny bias DMA warm
    the clock gate; "warmkeeper" matmuls with staggered deps on the
    softmax-chain outputs keep the PE busy across the chain gap so the
    tail runs at 2.4 GHz.
  - log-softmax folded BEFORE the gather (one fused scalar_tensor_tensor
    per batch-half: logp = psum/64 - lse); gather drains are plain
    copies.
  - one-hot: idx replicated across partitions by DMA (fp16, fused with a
    host-built iota block) then 8 DVE is_equal ops in 2x mode.
  - single 4-slot PSUM pool (8 banks); drain engines are parity-matched
    (DVE/ACT by (p+bt)%2, linT copies split the same way) so each
    gather's input dep and its PSUM-slot WAR dep land on one semaphore.
  - output bf16, out-DMAs alternate between scalar and sync families.
  - _install_wait_split legalizes the few remaining multi-wait
    instructions (this walrus build: max one sync-wait/instruction).
"""

import numpy as np

B, P, Q, C, F = 256, 64, 128, 1000, 2048
NCORES = 8
PL = P // NCORES          # partitionings per core
KT = F // 128             # contraction k-tiles
KC = 4                    # k-tiles per W DMA chunk
SCALE = 64.0              # W pre-scale keeps fp8e4 out of subnormals
N_WARM = 4                # junk matmuls at t=0 (PE HAM warm-up); sized to
                          # fill the dead window before the first W chunk
                          # lands without delaying the main phase


def _build_nc():
    import concourse.bass as bass
    import concourse.tile as tile
    from concourse import mybir
    from contextlib import ExitStack

    F32 = mybir.dt.float32
    BF16 = mybir.dt.bfloat16
    FP16 = mybir.dt.float16
    FP8 = mybir.dt.float8e4
    AF = mybir.ActivationFunctionType
    ALU = mybir.AluOpType

    nc = bass.Bass()
    bias_d = nc.declare_dram_parameter("biasr", [1, PL * Q + 128], BF16,
                                       isOutput=False)
    xT_d = nc.declare_dram_parameter("xT", [128, KT // 2, 2, 256], FP8,
                                     isOutput=False)
    id_d = nc.declare_dram_parameter("ident", [128, 128], BF16,
                                     isOutput=False)
    wm_d = nc.declare_dram_parameter(
        "wm", [KT // KC, 128, KC // 2, 2, PL * Q], FP8, isOutput=False)
    # cols [0,1000) = iota (row q has value q), [1000+p*1000, ...) = idx[p]
    idx_d = nc.declare_dram_parameter("idxq", [128, (PL + 1) * C], FP16,
                                      isOutput=False)
    out_d = nc.declare_dram_parameter("out", [B, PL, C], BF16, isOutput=True)

    with ExitStack() as ctx:
        tc = ctx.enter_context(tile.TileContext(nc))
        sb = ctx.enter_context(tc.tile_pool(name="sb", bufs=1))
        # one pool, 4 slots x 2 banks = all 8 PSUM banks; warmup target,
        # lin_bt0/1, transpose targets and gather outputs all rotate here
        ps = ctx.enter_context(
            tc.tile_pool(name="ps", bufs=4, space=bass.MemorySpace.PSUM))

        def fresh(shape, dtype, tag):
            return sb.tile(shape, dtype, tag=tag, name=tag)

        # ---- input DMAs ----------------------------------------------
        # sync family: bias (warmup dep) -> x -> ident -> W chunks
        biasr = fresh([1, PL * Q + 128], BF16, "biasr")
        nc.sync.dma_start(out=biasr[:], in_=bias_d[:])
        xT = fresh([128, KT // 2, 2, 256], FP8, "xT")
        nc.sync.dma_start(out=xT[:], in_=xT_d[:])
        ident = fresh([128, 128], BF16, "ident")
        nc.sync.dma_start(out=ident[:], in_=id_d[:])
        wkc = []
        for j in range(KT // KC):
            t = fresh([128, KC // 2, 2, PL * Q], FP8, f"wk{j}")
            nc.sync.dma_start(out=t[:], in_=wm_d[j])
            wkc.append(t)
        # idx LAST: the 16 physical DMA engines are shared across queue
        # families, so an early idx transfer would preempt the x/W
        # stream that gates the main matmuls.  The one-hots are only
        # needed by the gather phase (~5us after the main phase ends).
        idx_sb = fresh([128, (PL + 1) * C], FP16, "idxq")
        nc.sync.dma_start(out=idx_sb[:, 0:5 * C], in_=idx_d[:, 0:5 * C])
        nc.sync.dma_start(out=idx_sb[:, 5 * C:], in_=idx_d[:, 5 * C:])

        # ---- PE warm-up: junk K=1 matmuls off the bias row -----------
        ones_ap = biasr[:, PL * Q:PL * Q + 128]
        wu_ps = ps.tile([128, 1024], F32, tag="ps", name="wu_ps")
        for _ in range(N_WARM):
            nc.tensor.matmul(wu_ps[:, 0:512], ones_ap, biasr[:, 0:512],
                             start=True, stop=True)

        # ---- one-hot per p: 2x-mode DVE is_equal against iota --------
        oh = []
        for p in range(PL):
            t = fresh([128, C], BF16, f"oh{p}")
            nc.vector.tensor_tensor(
                out=t[:], in0=idx_sb[:, (1 + p) * C:(2 + p) * C],
                in1=idx_sb[:, 0:C], op=ALU.is_equal)
            oh.append(t)

        # ---- main matmuls: psum_lin[b, (p,q)] ------------------------
        # DoubleRow fp8: k-tiles are paired (K=256 per matmul, 2 MACs/
        # cell/cycle) halving the matmul+LDWEIGHTS count.  xT is laid
        # out [128f, T, ko, 256b], W chunks [128f, tt, ko, 1024pq].
        DR = mybir.MatmulPerfMode.DoubleRow
        lin = [ps.tile([128, PL, 128], F32, tag="ps", name=f"lin{bt}")
               for bt in (0, 1)]
        for bt in (0, 1):
            for ch in (0, 1):
                nc.tensor.matmul(
                    lin[bt][:, ch * 4:(ch + 1) * 4, :],
                    ones_ap, biasr[:, ch * 512:(ch + 1) * 512],
                    start=True, stop=False)
        for t in range(KT // 2):
            j, tt = t // (KC // 2), t % (KC // 2)
            for bt in (0, 1):
                for ch in (0, 1):
                    nc.tensor.matmul(
                        lin[bt][:, ch * 4:(ch + 1) * 4, :],
                        xT[:, t, :, bt * 128:(bt + 1) * 128],
                        wkc[j][:, tt, :, ch * 512:(ch + 1) * 512],
                        start=False, stop=(t == KT // 2 - 1),
                        perf_mode=DR)

        # ---- per-bt softmax chain (+ staggered PE warmkeepers) -------
        logpY, exps_t = [], []
        for bt in (0, 1):
            exps = fresh([128, PL, 128], BF16, f"exps{bt}")
            nc.scalar.activation(out=exps[:], in_=lin[bt][:], func=AF.Exp,
                                 scale=1.0 / SCALE)
            exps_t.append(exps)
            # tiny PE pings gated on fresh chain tensors: the HAM MID
            # window only re-throttles after ~3.4us of PE idle, so a
            # ~60-160ns matmul per chain stage keeps the clock warm
            nc.tensor.matmul(wu_ps[:, 0:128], exps[:, 7, :],
                             exps[:, 0, :], start=True, stop=True)
            sums = fresh([128, PL], F32, f"sums{bt}")
            nc.vector.tensor_reduce(out=sums[:], in_=exps[:],
                                    axis=mybir.AxisListType.X, op=ALU.add)
            lse = fresh([128, PL], F32, f"lse{bt}")
            nc.scalar.activation(out=lse[:], in_=sums[:], func=AF.Ln)
            lp = fresh([128, PL, 128], BF16, f"logpY{bt}")
            nc.vector.scalar_tensor_tensor(
                out=lp[:], in0=lin[bt][:], scalar=1.0 / SCALE,
                in1=lse[:].unsqueeze(2).broadcast_to((128, PL, 128)),
                op0=ALU.mult, op1=ALU.subtract)
            logpY.append(lp)
            nc.tensor.matmul(wu_ps[:, 0:128], lp[:, 7, :],
                             lp[:, 0, :], start=True, stop=True)

        # ---- per-bt: transpose -> gather -> drain -> out DMA ---------
        # drain/linT engine parity: (p + bt) even -> DVE, odd -> ACT;
        # PSUM slots rotate with stride 4 so a gather's WAR partner was
        # drained by its own input engine -> single-semaphore waits
        def drain_eng(bt, p):
            return nc.vector if (p + bt) % 2 == 0 else nc.scalar

        og = {}
        for bt in (0, 1):
            tr = ps.tile([128, PL, 128], BF16, tag="ps", name=f"tr{bt}")
            for p in range(PL):
                nc.tensor.transpose(tr[:, p, :], logpY[bt][:, p, :],
                                    ident[:])
            linT = fresh([128, PL, 128], BF16, f"linT{bt}")
            for par in (0, 1):
                eng = drain_eng(bt, par)
                src = tr[:, par::2, :]
                dst = linT[:, par::2, :]
                if eng is nc.vector:
                    nc.vector.tensor_copy(out=dst, in_=src)
                else:
                    nc.scalar.activation(out=dst, in_=src, func=AF.Copy)

            for p in range(PL):
                pair = p // 2
                po = ps.tile([128, 1024], F32, tag="ps", name=f"po{bt}_{p}")
                nc.tensor.matmul(po[:, 0:512], linT[:, p, :],
                                 oh[p][:, 0:512], start=True, stop=True)
                nc.tensor.matmul(po[:, 512:1000], linT[:, p, :],
                                 oh[p][:, 512:1000], start=True, stop=True)
                if p % 2 == 0:
                    og[(pair, bt)] = fresh([128, 2, C], BF16, f"og{pair}_{bt}")
                g = og[(pair, bt)]
                eng = drain_eng(bt, p)
                if eng is nc.vector:
                    nc.vector.tensor_copy(out=g[:, p % 2, :],
                                          in_=po[:, 0:1000])
                else:
                    nc.scalar.activation(out=g[:, p % 2, :],
                                         in_=po[:, 0:1000], func=AF.Copy)
                if p % 2 == 1:
                    bsl = slice(bt * 128, (bt + 1) * 128)
                    deng = nc.scalar if pair % 2 == 0 else nc.sync
                    deng.dma_start(out=out_d[bsl, p - 1:p + 1, :], in_=g[:])

    _install_wait_split(nc)
    return nc


def _install_wait_split(nc):
    """This walrus build encodes at most ONE sync-wait per instruction.
    Legalize at serialization time: any instruction carrying N>1 waits
    gets N-1 wait-only Drain instructions (same engine, so the queue
    stalls identically) inserted in front of it; the instruction keeps
    the last wait.  Semantically identical (serial sem waits)."""
    import json

    orig = nc.to_json_bytes

    def patched():
        m = json.loads(orig())
        for fn in m["functions"]:
            for bb in fn["blocks"]:
                out = []
                for inst in bb["instructions"]:
                    si = inst.get("sync_info")
                    waits = (si or {}).get("on_wait") or []
                    if len(waits) > 1:
                        head, keep = waits[:-1], waits[-1:]
                        for j, w in enumerate(head):
                            out.append({
                                "engine": inst["engine"],
                                "ins": [],
                                "outs": [],
                                "name": f"{inst['name']}-ws{j}",
                                "opcode": "Drain",
                                "sync_info": {
                                    "on_wait": [w],
                                    "on_update": [],
                                },
                            })
                        si["on_wait"] = keep
                    out.append(inst)
                bb["instructions"] = out
        return json.dumps(m).encode()

    nc.to_json_bytes = patched


def _host_inputs(x, W, b, part_idx):
    import ml_dtypes

    f8 = ml_dtypes.float8_e4m3
    bf = ml_dtypes.bfloat16

    # xT[f_sub, t, ko, b] = x[b, (2t+ko)*128 + f_sub]  (DoubleRow pairs)
    xT = np.ascontiguousarray(
        x.reshape(B, KT, 128).transpose(2, 1, 0)       # [128, KT, B]
        .reshape(128, KT // 2, 2, B)).astype(f8)
    ident = np.eye(128, dtype=np.float32).astype(bf)
    iota = np.arange(128, dtype=np.float16)

    in_maps = []
    for i in range(NCORES):
        sl = slice(i * PL, (i + 1) * PL)
        # wm[j, f_sub, tt, ko, p*128+q] = SCALE * W[p, q, k*128+f_sub]
        # with k = j*KC + 2*tt + ko  (DoubleRow pairs)
        wm = np.ascontiguousarray(
            (W[sl] * SCALE).transpose(2, 0, 1)          # [F, PL, Q]
            .reshape(KT // KC, KC, 128, PL * Q)
            .transpose(0, 2, 1, 3)                      # [J, 128, KC, PL*Q]
            .reshape(KT // KC, 128, KC // 2, 2, PL * Q)).astype(f8)
        biasr = np.empty((1, PL * Q + 128), dtype=bf)
        biasr[0, :PL * Q] = (b[sl] * SCALE).reshape(-1).astype(bf)
        biasr[0, PL * Q:] = 1.0
        idxq = np.empty((128, (PL + 1) * C), dtype=np.float16)
        idxq[:, 0:C] = iota[:, None]
        idxq[:, C:] = np.broadcast_to(
            part_idx[sl].astype(np.float16).reshape(1, PL * C),
            (128, PL * C))
        in_maps.append({"xT": xT, "biasr": biasr, "ident": ident,
                        "wm": wm, "idxq": idxq})
    return in_maps


def kernel(x, W, b, part_idx, _trace=False):
    from concourse.bass_utils import run_bass_kernel_spmd

    x = np.asarray(x, dtype=np.float32)
    W = np.asarray(W, dtype=np.float32)
    b = np.asarray(b, dtype=np.float32)
    part_idx = np.asarray(part_idx)

    nc = _build_nc()
    in_maps = _host_inputs(x, W, b, part_idx)
    res = run_bass_kernel_spmd(nc, in_maps, list(range(NCORES)),
                               trace=_trace)
    out = np.concatenate(
        [np.asarray(r["out"], dtype=np.float32) for r in res.results], axis=1)
    if _trace:
        return out, res
    return out


# revision 22
# speedup vs baseline: 1.0164x; 1.0164x over previous
"""Trainium2 Bass kernel for nn_CombinatorialClassifier (v4).

Computation (reference):
    logits = einsum('bf,pqf->bpq', x, W) + b        # [B,P,Q]
    logp   = log_softmax(logits, axis=2)            # [B,P,Q]
    out    = take_along_axis(logp, part_idx, 2)     # [B,P,C]

Shapes: B=256, P=64, Q=128, C=1000, F=2048.  Expert-parallel over P
across 8 cores (PL=8 partitionings per core), no collectives.

Structure:
  - main matmul in "b-orientation": stationary = xT k-slab [128f, 128b]
    (fp8e4), moving = W k-slab [128f, (p,q)-chunk] (fp8e4, x64 scale
    folded back out in the softmax) -> psum_lin[b, (p,q)].  W streams
    from HBM in 4 chunks on the sync HWDGE family; idx streams in
    parallel on the scalar HWDGE family so it does not delay W.
  - PE HAM warm-up: junk K=1 matmuls at t=0 off the tiny bias DMA warm
    the clock gate; "warmkeeper" matmuls with staggered deps on the
    softmax-chain outputs keep the PE busy across the chain gap so the
    tail runs at 2.4 GHz.
  - log-softmax folded BEFORE the gather (one fused scalar_tensor_tensor
    per batch-half: logp = psum/64 - lse); gather drains are plain
    copies.
  - one-hot: idx replicated across partitions by DMA (fp16, fused with a
    host-built iota block) then 8 DVE is_equal ops in 2x mode.
  - single 4-slot PSUM pool (8 banks); drain engines are parity-matched
    (DVE/ACT by (p+bt)%2, linT copies split the same way) so each
    gather's input dep and its PSUM-slot WAR dep land on one semaphore.
  - output bf16, out-DMAs alternate between scalar and sync families.
  - _install_wait_split legalizes the few remaining multi-wait
    instructions (this walrus build: max one sync-wait/instruction).
"""

import numpy as np

B, P, Q, C, F = 256, 64, 128, 1000, 2048
NCORES = 8
PL = P // NCORES          # partitionings per core
KT = F // 128             # contraction k-tiles
KC = 4                    # k-tiles per W DMA chunk
SCALE = 64.0              # W pre-scale keeps fp8e4 out of subnormals
N_WARM = 7                # junk matmuls at t=0 (PE HAM warm-up); sized to
                          # fill the dead window before the first W chunk
                          # lands without delaying the main phase


def _build_nc():
    import concourse.bass as bass
    import concourse.tile as tile
    from concourse import mybir
    from contextlib import ExitStack

    F32 = mybir.dt.float32
    BF16 = mybir.dt.bfloat16
    FP16 = mybir.dt.float16
    FP8 = mybir.dt.float8e4
    AF = mybir.ActivationFunctionType
    ALU = mybir.AluOpType

    nc = bass.Bass()
    bias_d = nc.declare_dram_parameter("biasr", [1, PL * Q + 128], BF16,
                                       isOutput=False)
    xT_d = nc.declare_dram_parameter("xT", [128, KT // 2, 2, 256], FP8,
                                     isOutput=False)
    id_d = nc.declare_dram_parameter("ident", [128, 128], BF16,
                                     isOutput=False)
    wm_d = nc.declare_dram_parameter(
        "wm", [KT // KC, 128, KC // 2, 2, PL * Q], FP8, isOutput=False)
    # cols [0,1000) = iota (row q has value q), [1000+p*1000, ...) = idx[p]
    idx_d = nc.declare_dram_parameter("idxq", [128, (PL + 1) * C], FP16,
                                      isOutput=False)
    out_d = nc.declare_dram_parameter("out", [B, PL, C], BF16, isOutput=True)

    with ExitStack() as ctx:
        tc = ctx.enter_context(tile.TileContext(nc))
        sb = ctx.enter_context(tc.tile_pool(name="sb", bufs=1))
        # one pool, 4 slots x 2 banks = all 8 PSUM banks; warmup target,
        # lin_bt0/1, transpose targets and gather outputs all rotate here
        ps = ctx.enter_context(
            tc.tile_pool(name="ps", bufs=4, space=bass.MemorySpace.PSUM))

        def fresh(shape, dtype, tag):
            return sb.tile(shape, dtype, tag=tag, name=tag)

        # ---- input DMAs ----------------------------------------------
        # sync family: bias (warmup dep) -> W chunks -> idx; x and ident
        # ride the scalar family so W streams as early as possible
        biasr = fresh([1, PL * Q + 128], BF16, "biasr")
        nc.sync.dma_start(out=biasr[:], in_=bias_d[:])
        xT = fresh([128, KT // 2, 2, 256], FP8, "xT")
        nc.scalar.dma_start(out=xT[:], in_=xT_d[:])
        ident = fresh([128, 128], BF16, "ident")
        nc.scalar.dma_start(out=ident[:], in_=id_d[:])
        wkc = []
        for j in range(KT // KC):
            t = fresh([128, KC // 2, 2, PL * Q], FP8, f"wk{j}")
            nc.sync.dma_start(out=t[:], in_=wm_d[j])
            wkc.append(t)
        # idx LAST: the 16 physical DMA engines are shared across queue
        # families, so an early idx transfer would preempt the x/W
        # stream that gates the main matmuls.  The one-hots are only
        # needed by the gather phase (~5us after the main phase ends).
        idx_sb = fresh([128, (PL + 1) * C], FP16, "idxq")
        nc.sync.dma_start(out=idx_sb[:, 0:5 * C], in_=idx_d[:, 0:5 * C])
        nc.sync.dma_start(out=idx_sb[:, 5 * C:], in_=idx_d[:, 5 * C:])

        # ---- PE warm-up: junk K=1 matmuls off the bias row -----------
        ones_ap = biasr[:, PL * Q:PL * Q + 128]
        wu_ps = ps.tile([128, 1024], F32, tag="ps", name="wu_ps")
        for _ in range(N_WARM):
            nc.tensor.matmul(wu_ps[:, 0:512], ones_ap, biasr[:, 0:512],
                             start=True, stop=True)

        # ---- one-hot per p: 2x-mode DVE is_equal against iota --------
        oh = []
        for p in range(PL):
            t = fresh([128, C], BF16, f"oh{p}")
            nc.vector.tensor_tensor(
                out=t[:], in0=idx_sb[:, (1 + p) * C:(2 + p) * C],
                in1=idx_sb[:, 0:C], op=ALU.is_equal)
            oh.append(t)

        # ---- main matmuls: psum_lin[b, (p,q)] ------------------------
        # DoubleRow fp8: k-tiles are paired (K=256 per matmul, 2 MACs/
        # cell/cycle) halving the matmul+LDWEIGHTS count.  xT is laid
        # out [128f, T, ko, 256b], W chunks [128f, tt, ko, 1024pq].
        # bt-OUTER: bt0's accumulation completes ~5us before bt1's, so
        # its softmax chain and tail overlap bt1's main matmuls on the
        # PE FIFO (also keeps the PE continuously busy -> HAM stays
        # warm with no keeper matmuls)
        DR = mybir.MatmulPerfMode.DoubleRow
        lin = [ps.tile([128, PL, 128], F32, tag="ps", name=f"lin{bt}")
               for bt in (0, 1)]
        logpY = [None, None]

        def emit_main(bt):
            for ch in (0, 1):
                nc.tensor.matmul(
                    lin[bt][:, ch * 4:(ch + 1) * 4, :],
                    ones_ap, biasr[:, ch * 512:(ch + 1) * 512],
                    start=True, stop=False)
            for t in range(KT // 2):
                j, tt = t // (KC // 2), t % (KC // 2)
                for ch in (0, 1):
                    nc.tensor.matmul(
                        lin[bt][:, ch * 4:(ch + 1) * 4, :],
                        xT[:, t, :, bt * 128:(bt + 1) * 128],
                        wkc[j][:, tt, :, ch * 512:(ch + 1) * 512],
                        start=False, stop=(t == KT // 2 - 1),
                        perf_mode=DR)

        def emit_chain(bt):
            exps = fresh([128, PL, 128], BF16, f"exps{bt}")
            nc.scalar.activation(out=exps[:], in_=lin[bt][:], func=AF.Exp,
                                 scale=1.0 / SCALE)
            sums = fresh([128, PL], F32, f"sums{bt}")
            nc.vector.tensor_reduce(out=sums[:], in_=exps[:],
                                    axis=mybir.AxisListType.X, op=ALU.add)
            lse = fresh([128, PL], F32, f"lse{bt}")
            nc.scalar.activation(out=lse[:], in_=sums[:], func=AF.Ln)
            lp = fresh([128, PL, 128], BF16, f"logpY{bt}")
            nc.vector.scalar_tensor_tensor(
                out=lp[:], in0=lin[bt][:], scalar=1.0 / SCALE,
                in1=lse[:].unsqueeze(2).broadcast_to((128, PL, 128)),
                op0=ALU.mult, op1=ALU.subtract)
            logpY[bt] = lp

        emit_main(0)
        emit_chain(0)
        emit_main(1)
        emit_chain(1)

        # ---- per-bt: transpose -> gather -> drain -> out DMA ---------
        # drain/linT engine parity: (p + bt) even -> DVE, odd -> ACT;
        # PSUM slots rotate with stride 4 so a gather's WAR partner was
        # drained by its own input engine -> single-semaphore waits
        def drain_eng(bt, p):
            return nc.vector if (p + bt) % 2 == 0 else nc.scalar

        og = {}
        for bt in (0, 1):
            tr = ps.tile([128, PL, 128], BF16, tag="ps", name=f"tr{bt}")
            for p in range(PL):
                nc.tensor.transpose(tr[:, p, :], logpY[bt][:, p, :],
                                    ident[:])
            linT = fresh([128, PL, 128], BF16, f"linT{bt}")
            for par in (0, 1):
                eng = drain_eng(bt, par)
                src = tr[:, par::2, :]
                dst = linT[:, par::2, :]
                if eng is nc.vector:
                    nc.vector.tensor_copy(out=dst, in_=src)
                else:
                    nc.scalar.activation(out=dst, in_=src, func=AF.Copy)

            for p in range(PL):
                pair = p // 2
                po = ps.tile([128, 1024], F32, tag="ps", name=f"po{bt}_{p}")
                nc.tensor.matmul(po[:, 0:512], linT[:, p, :],
                                 oh[p][:, 0:512], start=True, stop=True)
                nc.tensor.matmul(po[:, 512:1000], linT[:, p, :],
                                 oh[p][:, 512:1000], start=True, stop=True)
                if p % 2 == 0:
                    og[(pair, bt)] = fresh([128, 2, C], BF16, f"og{pair}_{bt}")
                g = og[(pair, bt)]
                eng = drain_eng(bt, p)
                if eng is nc.vector:
                    nc.vector.tensor_copy(out=g[:, p % 2, :],
                                          in_=po[:, 0:1000])
                else:
                    nc.scalar.activation(out=g[:, p % 2, :],
                                         in_=po[:, 0:1000], func=AF.Copy)
                if p % 2 == 1:
                    bsl = slice(bt * 128, (bt + 1) * 128)
                    deng = nc.scalar if pair % 2 == 0 else nc.sync
                    deng.dma_start(out=out_d[bsl, p - 1:p + 1, :], in_=g[:])

    _install_wait_split(nc)
    return nc


def _install_wait_split(nc):
    """This walrus build encodes at most ONE sync-wait per instruction.
    Legalize at serialization time: any instruction carrying N>1 waits
    gets N-1 wait-only Drain instructions (same engine, so the queue
    stalls identically) inserted in front of it; the instruction keeps
    the last wait.  Semantically identical (serial sem waits)."""
    import json

    orig = nc.to_json_bytes

    def patched():
        m = json.loads(orig())
        for fn in m["functions"]:
            for bb in fn["blocks"]:
                out = []
                for inst in bb["instructions"]:
                    si = inst.get("sync_info")
                    waits = (si or {}).get("on_wait") or []
                    if len(waits) > 1:
                        head, keep = waits[:-1], waits[-1:]
                        for j, w in enumerate(head):
                            out.append({
                                "engine": inst["engine"],
                                "ins": [],
                                "outs": [],
                                "name": f"{inst['name']}-ws{j}",
                                "opcode": "Drain",
                                "sync_info": {
                                    "on_wait": [w],
                                    "on_update": [],
                                },
                            })
                        si["on_wait"] = keep
                    out.append(inst)
                bb["instructions"] = out
        return json.dumps(m).encode()

    nc.to_json_bytes = patched


def _host_inputs(x, W, b, part_idx):
    import ml_dtypes

    f8 = ml_dtypes.float8_e4m3
    bf = ml_dtypes.bfloat16

    # xT[f_sub, t, ko, b] = x[b, (2t+ko)*128 + f_sub]  (DoubleRow pairs)
    xT = np.ascontiguousarray(
        x.reshape(B, KT, 128).transpose(2, 1, 0)       # [128, KT, B]
        .reshape(128, KT // 2, 2, B)).astype(f8)
    ident = np.eye(128, dtype=np.float32).astype(bf)
    iota = np.arange(128, dtype=np.float16)

    in_maps = []
    for i in range(NCORES):
        sl = slice(i * PL, (i + 1) * PL)
        # wm[j, f_sub, tt, ko, p*128+q] = SCALE * W[p, q, k*128+f_sub]
        # with k = j*KC + 2*tt + ko  (DoubleRow pairs)
        wm = np.ascontiguousarray(
            (W[sl] * SCALE).transpose(2, 0, 1)          # [F, PL, Q]
            .reshape(KT // KC, KC, 128, PL * Q)
            .transpose(0, 2, 1, 3)                      # [J, 128, KC, PL*Q]
            .reshape(KT // KC, 128, KC // 2, 2, PL * Q)).astype(f8)
        biasr = np.empty((1, PL * Q + 128), dtype=bf)
        biasr[0, :PL * Q] = (b[sl] * SCALE).reshape(-1).astype(bf)
        biasr[0, PL * Q:] = 1.0
        idxq = np.empty((128, (PL + 1) * C), dtype=np.float16)
        idxq[:, 0:C] = iota[:, None]
        idxq[:, C:] = np.broadcast_to(
            part_idx[sl].astype(np.float16).reshape(1, PL * C),
            (128, PL * C))
        in_maps.append({"xT": xT, "biasr": biasr, "ident": ident,
                        "wm": wm, "idxq": idxq})
    return in_maps


def kernel(x, W, b, part_idx, _trace=False):
    from concourse.bass_utils import run_bass_kernel_spmd

    x = np.asarray(x, dtype=np.float32)
    W = np.asarray(W, dtype=np.float32)
    b = np.asarray(b, dtype=np.float32)
    part_idx = np.asarray(part_idx)

    nc = _build_nc()
    in_maps = _host_inputs(x, W, b, part_idx)
    res = run_bass_kernel_spmd(nc, in_maps, list(range(NCORES)),
                               trace=_trace)
    out = np.concatenate(
        [np.asarray(r["out"], dtype=np.float32) for r in res.results], axis=1)
    if _trace:
        return out, res
    return out
